# revision 55
# baseline (speedup 1.0000x reference)
"""Trainium2 Bass kernel for nn_AutodiffChannel: 6-biquad EQ cascade over
(64, 1, 262144) fp32 audio, data-parallel over 8 NeuronCores.

Algorithm (per sequence, LTI block-state decomposition):
  The 6-stage DF2T biquad cascade is a 12-state linear system
  s' = A s + B x, y = C s + D x.  Split T=262144 into 2048 chunks of
  L=128.  Then per chunk c:
      y_c = Phi x_c + Gamma S_c          (Phi  = 128x128 lower-tri Toeplitz
                                          of the impulse response h[0:128],
                                          Gamma[m,:] = C A^m)
      U_c = M x_c                        (M[:,n] = A^(127-n) B)
      S_c = sum_{j<c} (A^128)^(c-1-j) U_j   (exclusive prefix "state scan")
  The prefix is computed with a Kogge-Stone scan (11 levels) using
  precomputed powers P_d = (A^128)^(2^d).  The tiny per-sequence setup
  (h, Gamma, M, P_d) is computed host-side in float64.

Device dataflow per core (8 sequences), lean-precision variant:
  x arrives as bf16 (hi part only; ~2e-3 relative error, inside the 2e-2
  budget).  U = M x uses a 2-term bf16 weight split and accumulates all 8
  seqs into one 96-row PSUM tile per 512-column block.  The prefix scan
  stays fp32 and runs as a radix-4 Brent-Kung (up/down sweeps, ~2N matmul
  columns, 11 serial levels); the scan-independent FIR term Th x is
  spliced into the scan's serial gaps to keep the PE busy and spilled to
  an fp32 SBUF buffer.  S is split into bf16 hi/lo; the correction
  Gamma S uses 3 bf16 terms (Gh Sh, Gl Sh, Gh Sl) fused with the spilled
  FIR part and the bf16 cast in one DVE (or Act+Pool) op.  Total rel err
  ~3.9e-3.  x/y/weights each move in a single DMA with 32 KiB contiguous
  DRAM per partition row (DMA descriptor rate, ~0.6 us/descriptor/queue,
  dominates small transfers).  y returns in chunk-column layout as bf16;
  the host does the final (free) transpose + fp32 cast.
"""
import sys

for _p in ("/opt/trn_rl_repo", "/opt/trn_rl_repo/concourse"):
    if _p not in sys.path:
        sys.path.insert(0, _p)

import numpy as np

import concourse.bacc as bacc
import concourse.mybir as mybir
from concourse.tile import TileContext
from concourse.bass_utils import run_bass_kernel_spmd  # noqa: F401 (env check)

# ---------------------------------------------------------------- problem dims
B, C, T = 64, 1, 262144
N_CORES = 8
SEQ_PER_CORE = B * C // N_CORES  # 8
L = 128                     # chunk length
NCH = T // L                # 2048 chunks per sequence
ROWS = 128                  # partitions: within-chunk sample index
COLS = NCH                  # 2048 chunk columns
LEVELS = 11                 # ceil(log2(NCH))
NSTATE = 12
BLK = 512                   # column blocking (1 PSUM bank of fp32)
NBLK = COLS // BLK
RJ = 5                      # radix-4 scan levels (4^5 = 1024, x2 top)
NSL = LEVELS + RJ           # scanP slots: 11 powers of 2 + 5 "3*4^j" powers
F32 = mybir.dt.float32
BF16 = mybir.dt.bfloat16
F16 = mybir.dt.float16

PARAM_RANGES = np.array([
    [-24.0, 24.0], [20.0, 200.0], [0.1, 10.0],
    [-24.0, 24.0], [200.0, 2000.0], [0.1, 10.0],
    [-24.0, 24.0], [200.0, 2000.0], [0.1, 10.0],
    [-24.0, 24.0], [2000.0, 8000.0], [0.1, 10.0],
    [-24.0, 24.0], [4000.0, 12000.0], [0.1, 10.0],
    [-24.0, 24.0], [4000.0, 12000.0], [0.1, 10.0],
], dtype=np.float32)
FILTER_TYPES = ["low_shelf", "peaking", "peaking", "peaking", "peaking",
                "high_shelf"]


# ------------------------------------------------------------- host-side setup
def _sigmoid_f32(z):
    z = z.astype(np.float32)
    out = np.empty_like(z)
    pos = z >= 0
    out[pos] = (np.float32(1.0) / (np.float32(1.0) + np.exp(-z[pos]))).astype(
        np.float32)
    ez = np.exp(z[~pos]).astype(np.float32)
    out[~pos] = (ez / (np.float32(1.0) + ez)).astype(np.float32)
    return out


def _biquad_coeffs_f32(g, f, q, sr, ftype):
    """fp32-faithful audio-EQ-cookbook coefficients (matches reference)."""
    f32 = np.float32
    A = np.power(f32(10.0), (g / f32(40.0)).astype(f32)).astype(f32)
    w0 = (f32(2.0) * f32(np.pi) * (f / f32(sr))).astype(f32)
    alpha = (np.sin(w0, dtype=f32) / (f32(2.0) * q)).astype(f32)
    c = np.cos(w0, dtype=f32)
    sA = np.sqrt(A).astype(f32)
    one, two = f32(1.0), f32(2.0)
    if ftype == "low_shelf":
        b0 = A * ((A + one) - (A - one) * c + two * sA * alpha)
        b1 = two * A * ((A - one) - (A + one) * c)
        b2 = A * ((A + one) - (A - one) * c - two * sA * alpha)
        a0 = (A + one) + (A - one) * c + two * sA * alpha
        a1 = -two * ((A - one) + (A + one) * c)
        a2 = (A + one) + (A - one) * c - two * sA * alpha
    elif ftype == "high_shelf":
        b0 = A * ((A + one) + (A - one) * c + two * sA * alpha)
        b1 = -two * A * ((A - one) + (A + one) * c)
        b2 = A * ((A + one) + (A - one) * c - two * sA * alpha)
        a0 = (A + one) - (A - one) * c + two * sA * alpha
        a1 = two * ((A - one) - (A + one) * c)
        a2 = (A + one) - (A - one) * c - two * sA * alpha
    else:
        b0 = one + alpha * A
        b1 = -two * c
        b2 = one - alpha * A
        a0 = one + alpha / A
        a1 = -two * c
        a2 = one - alpha / A
    bc = (np.stack([b0, b1, b2], -1).astype(f32) / a0[..., None]).astype(f32)
    ac = (np.stack([a0, a1, a2], -1).astype(f32) / a0[..., None]).astype(f32)
    return bc, ac


def _coeffs_from_inputs(p, W, b, sample_rate):
    z = (p.astype(np.float32) @ W.astype(np.float32).T
         + b.astype(np.float32)).astype(np.float32)
    pn = _sigmoid_f32(z)
    lo, hi = PARAM_RANGES[:, 0], PARAM_RANGES[:, 1]
    params = (pn * (hi - lo) + lo).astype(np.float32)
    bcs, acs = [], []
    for k, ftype in enumerate(FILTER_TYPES):
        bc, ac = _biquad_coeffs_f32(
            params[:, 3 * k], params[:, 3 * k + 1], params[:, 3 * k + 2],
            float(sample_rate), ftype)
        bcs.append(bc)
        acs.append(ac)
    return np.stack(bcs), np.stack(acs)  # (6, B, 3) fp32


def _state_space(bc, ac):
    """Vectorized float64 (A, B, C, D) per sequence from fp32 DF2T coeffs."""
    nb = bc.shape[1]
    bc64 = bc.astype(np.float64)
    ac64 = ac.astype(np.float64)

    def step(s, x):
        s = s.copy()
        v = x
        for k in range(6):
            b0, b1, b2 = bc64[k, :, 0], bc64[k, :, 1], bc64[k, :, 2]
            a1, a2 = ac64[k, :, 1], ac64[k, :, 2]
            s1, s2 = s[:, 2 * k], s[:, 2 * k + 1]
            y = b0 * v + s1
            s[:, 2 * k] = b1 * v - a1 * y + s2
            s[:, 2 * k + 1] = b2 * v - a2 * y
            v = y
        return s, v

    A = np.zeros((nb, NSTATE, NSTATE))
    Cv = np.zeros((nb, NSTATE))
    for i in range(NSTATE):
        e = np.zeros((nb, NSTATE))
        e[:, i] = 1.0
        sp, y = step(e, np.zeros(nb))
        A[:, :, i] = sp
        Cv[:, i] = y
    Bv, D = step(np.zeros((nb, NSTATE)), np.ones(nb))
    return A, Bv, Cv, D


def _derived(A, Bv, Cv, D):
    """h (nb,L), Gamma (nb,L,12), M (nb,12,L), Pd (nb,LEVELS,12,12) in f64."""
    nb = A.shape[0]
    h = np.zeros((nb, L))
    Gam = np.zeros((nb, L, NSTATE))
    M = np.zeros((nb, NSTATE, L))
    h[:, 0] = D
    cam = Cv.copy()          # C A^m
    amb = Bv.copy()          # A^m B
    for m in range(L):
        Gam[:, m, :] = cam
        M[:, :, L - 1 - m] = amb
        if m + 1 < L:
            h[:, m + 1] = np.einsum("bi,bi->b", cam, Bv)
        cam = np.einsum("bi,bij->bj", cam, A)
        amb = np.einsum("bij,bj->bi", A, amb)
    sq = A.copy()
    for _ in range(7):       # A^(2^7) = A^128
        sq = sq @ sq
    Pd = np.zeros((nb, NSL, NSTATE, NSTATE))
    for d in range(LEVELS):
        Pd[:, d] = sq
        sq = sq @ sq
    for j in range(RJ):      # (A^128)^(3*4^j) for the radix-4 scan
        Pd[:, LEVELS + j] = Pd[:, 2 * j] @ Pd[:, 2 * j + 1]
    return h, Gam, M, Pd


def _split_hi_lo(a):
    """Split fp32 into bf16 hi + bf16 lo (a ~= hi + lo, ~17-bit mantissa)."""
    import ml_dtypes
    a = a.astype(np.float32)
    hi = a.astype(ml_dtypes.bfloat16)
    lo = (a - hi.astype(np.float32)).astype(ml_dtypes.bfloat16)
    return hi, lo


def _pack_weights(h, Gam, M, Pd):
    """fp32 device weight tensors, per core."""
    nb = h.shape[0]
    m_idx = np.arange(L)
    diff = m_idx[None, :] - m_idx[:, None]          # [n, m] = m - n
    toepT = np.where(diff >= 0, h[:, np.clip(diff, 0, L - 1)],
                     0.0).astype(np.float32)        # (nb, n=128, m=128)
    # mT/gammaT embedded at per-seq 12-row offsets inside a 96-row frame so
    # every device access stays at base partition 0
    gammaT = np.zeros((nb, 96, L), np.float32)         # (nb, k-embed, m)
    mT = np.zeros((nb, L, 96), np.float32)             # (nb, n, k-embed)
    for g in range(nb):
        s8 = g % SEQ_PER_CORE
        gammaT[g, 12 * s8:12 * s8 + 12, :] = Gam[g].T.astype(np.float32)
        mT[g, :, 12 * s8:12 * s8 + 12] = M[g].T.astype(np.float32)
    scanP = np.zeros((N_CORES, NSL, 96, 96), np.float32)
    for core in range(N_CORES):
        for s in range(SEQ_PER_CORE):
            g = core * SEQ_PER_CORE + s
            for d in range(NSL):
                scanP[core, d, 12 * s:12 * s + 12, 12 * s:12 * s + 12] = \
                    Pd[g, d].T.astype(np.float32)
    return toepT, gammaT, mT, scanP


# ------------------------------------------------------------ device kernel IR
_NC_CACHE = {}


def build_nc(rep=1, ablate=""):
    key = (rep, ablate)
    if key in _NC_CACHE:
        return _NC_CACHE[key]
    nc = bacc.Bacc("TRN2")
    # DMA descriptor rate (~0.6us per descriptor per queue) dominates small
    # transfers, so x/y/bf16-weights each use ONE dma with 32 KiB contiguous
    # DRAM per partition row: layout [row, blk, seq, col]
    xh_d = nc.dram_tensor("xh", [ROWS, NBLK, SEQ_PER_CORE, BLK], F16,
                          kind="ExternalInput")
    # packed fp16 weights (single terms): cols [0:768) mT, [768:1792)
    # toepT, [1792:2816) gammaT (rows 96..128 zero-padded)
    wpk_d = nc.dram_tensor("wpk", [ROWS, 2816], F16, kind="ExternalInput")
    scanP_d = nc.dram_tensor("scanP", [96, NSL * 96], F32,
                             kind="ExternalInput")
    y_d = nc.dram_tensor("y", [ROWS, NBLK, SEQ_PER_CORE, BLK], BF16,
                         kind="ExternalOutput")

    with TileContext(nc) as tc:
        with tc.tile_pool(name="weights", bufs=1) as wpool:
            wpk_sb = wpool.tile([ROWS, 2816], F16)
            scanP_sb = wpool.tile([96, NSL * 96], F32)
            mT_sb = wpk_sb[:, 0:768]
            toepT_sb = wpk_sb[:, 768:1792]
            gammaT_sb = wpk_sb[0:96, 1792:2816]

            def dma_weights():
                nc.sync.dma_start(out=wpk_sb, in_=wpk_d[:])
                nc.sync.dma_start(out=scanP_sb, in_=scanP_d[:])

            with tc.tile_pool(name="xt", bufs=1) as xtpool, \
                 tc.tile_pool(name="ysb", bufs=1) as ypool:
                for r in range(rep):
                    _one_pass(nc, tc, xh_d, y_d, toepT_sb, gammaT_sb,
                              mT_sb, scanP_sb, xtpool, ypool, ablate,
                              dma_weights if r == 0 else None)
    nc.compile()
    _NC_CACHE[key] = nc
    return nc


def _one_pass(nc, tc, xh_d, y_d, toepT_sb, gammaT_sb, mT_sb, scanP_sb,
              xtpool, ypool, ablate="", dma_weights=None):
    # single x tile, column index = blk*4096 + sq*512 + j; two half DMAs
    # (16 KiB per partition row) so phase A starts after the first half
    XT = xtpool.tile([ROWS, SEQ_PER_CORE * COLS], F16, tag="xt", name="xt")
    nc.sync.dma_start(out=XT[:, 0:2 * 8 * BLK], in_=xh_d[:, 0:2])
    nc.sync.dma_start(out=XT[:, 2 * 8 * BLK:], in_=xh_d[:, 2:4])
    if dma_weights is not None:
        dma_weights()

    def xsl(sq, i):
        off = (i * SEQ_PER_CORE + sq) * BLK
        return XT[:, off:off + BLK]

    def wsl(base, h, sq, width):     # hi/lo weight slice helper
        off = (h * SEQ_PER_CORE + sq) * width
        return base[:, off:off + width]

    with tc.tile_pool(name="wbuf", bufs=1) as wbpool:
        # scan state (in place): col 0 = zeros, col 1+c = U_c then the
        # inclusive prefix W_c; rows 12s..12s+12 = seq s
        wb = wbpool.tile([96, NCH + 1], F32, tag="wb")
        # fp32 spill of the scan-independent FIR part Phi x (per seq)
        yx = wbpool.tile([ROWS, SEQ_PER_CORE * COLS], F32, tag="yx")
        nc.gpsimd.memset(wb[:, 0:1], 0.0)

        # ---- phase A: U_c = M x_c, all 8 seqs into one 96-row tile ----
        with tc.tile_pool(name="up", bufs=2, space="PSUM") as upsum:
            for i in range(NBLK):
                up = upsum.tile([96, BLK], F32, tag="up")
                for sq in range(SEQ_PER_CORE):
                        nc.tensor.matmul(
                            up[:], lhsT=mT_sb[:, sq * 96:(sq + 1) * 96],
                            rhs=xsl(sq, i),
                            start=(sq == 0), stop=(sq == SEQ_PER_CORE - 1))
                # Pool/GpSimd cannot read PSUM: copies must use DVE or Act
                if i % 2 == 0:
                    nc.vector.tensor_copy(
                        out=wb[:, 1 + i * BLK:1 + (i + 1) * BLK], in_=up[:])
                else:
                    nc.scalar.copy(wb[:, 1 + i * BLK:1 + (i + 1) * BLK],
                                   up[:])

        if ablate == "A":
            return

        with tc.tile_pool(name="shl", bufs=1) as shlpool, \
             tc.tile_pool(name="sp", bufs=3, space="PSUM") as spsum, \
             tc.tile_pool(name="ytp", bufs=4, space="PSUM") as ytpsum:
            # fp16 S (single rounding; fp16 range covers |S| ~ 3e3)
            SHL = shlpool.tile([96, NCH], F16, tag="shl")
            # wb cols 1..2048 as a scan array W[1..N]
            wbv = wb[:, 1:NCH + 1]

            # seq-major: consecutive fx matmuls share the same Th
            # stationary operand (cheaper weight reloads)
            fx_jobs = [(sq, i) for sq in range(SEQ_PER_CORE)
                       for i in range(NBLK)]
            fx_done = [0]

            def xsl_yx(sq, i):
                off = (i * SEQ_PER_CORE + sq) * BLK
                return yx[:, off:off + BLK]

            def emit_fx(n):
                # Phi x term (Th only): independent of the scan; spliced
                # into the Brent-Kung serial gaps to keep the PE busy
                for _ in range(n):
                    if fx_done[0] >= len(fx_jobs):
                        return
                    sq, i = fx_jobs[fx_done[0]]
                    fx_done[0] += 1
                    ytp = ytpsum.tile([128, BLK], F32, tag="ytp")
                    nc.tensor.matmul(
                        ytp[:], lhsT=toepT_sb[:, sq * L:(sq + 1) * L],
                        rhs=xsl(sq, i), start=True, stop=True)
                    if (sq + i) % 2 == 0:
                        nc.vector.tensor_copy(out=xsl_yx(sq, i), in_=ytp[:])
                    else:
                        nc.scalar.copy(xsl_yx(sq, i), ytp[:])

            def pslice(d):
                return scanP_sb[:, d * 96:(d + 1) * 96]

            def stview(stride):
                return wbv.rearrange("p (k s) -> p k s", s=stride)

            def bk_add(sp, tgt, nk):
                nc.vector.tensor_add(
                    out=tgt, in0=tgt,
                    in1=sp[:, 0:nk].rearrange("p (n o) -> p n o", o=1))

            def up_level(j):
                # radix-4: W[g4Q+4Q] += P^3Q W[g4Q+Q] + P^2Q W[g4Q+2Q]
                #                       + P^Q W[g4Q+3Q]
                q = 4 ** j
                g4 = 4 * q
                ng = NCH // g4
                v = stview(g4)
                sp = spsum.tile([96, BLK], F32, tag="sp")
                srcs = ((LEVELS + j, q - 1), (2 * j + 1, 2 * q - 1),
                        (2 * j, 3 * q - 1))
                for k, (d, idx) in enumerate(srcs):
                    nc.tensor.matmul(sp[:, 0:ng], lhsT=pslice(d),
                                     rhs=v[:, 0:ng, idx:idx + 1],
                                     start=(k == 0), stop=(k == 2))
                bk_add(sp, v[:, 0:ng, g4 - 1:g4], ng)

            def down_level(j, half=None):
                # finalize W[g4Q+kQ] (k=1,2,3) from the complete prefix
                # W[g4Q] (g>=1) + the old Q-block partials
                q = 4 ** j
                g4 = 4 * q
                ng = NCH // g4
                v = stview(g4)
                lo, hi = 0, ng
                if half == 0:
                    hi = ng // 2
                elif half == 1:
                    lo = ng // 2
                n = hi - lo
                # ALL matmuls first (they must see pre-level "old" Q-block
                # partials), THEN the adds: emitting an add earlier would
                # make the later matmuls read the already-updated columns
                n1, lo1 = (n - 1, lo + 1) if lo == 0 else (n, lo)
                # k=1: src complete W[g4Q] only (g >= 1)
                sp1 = spsum.tile([96, BLK], F32, tag="sp")
                nc.tensor.matmul(sp1[:, 0:n1], lhsT=pslice(2 * j),
                                 rhs=v[:, lo1 - 1:lo1 - 1 + n1,
                                       g4 - 1:g4],
                                 start=True, stop=True)
                # k=2: P^2Q W[g4Q] (g>=1) + P^Q W_old[g4Q+Q]
                sp2 = spsum.tile([96, BLK], F32, tag="sp")
                nc.tensor.matmul(sp2[:, 0:n], lhsT=pslice(2 * j),
                                 rhs=v[:, lo:hi, q - 1:q],
                                 start=True, stop=False)
                nc.tensor.matmul(sp2[:, (lo1 - lo):(lo1 - lo) + n1],
                                 lhsT=pslice(2 * j + 1),
                                 rhs=v[:, lo1 - 1:lo1 - 1 + n1, g4 - 1:g4],
                                 start=False, stop=True)
                # k=3: P^3Q W[g4Q] (g>=1) + P^2Q W_old[+Q] + P^Q W_old[+2Q]
                sp3 = spsum.tile([96, BLK], F32, tag="sp")
                nc.tensor.matmul(sp3[:, 0:n], lhsT=pslice(2 * j + 1),
                                 rhs=v[:, lo:hi, q - 1:q],
                                 start=True, stop=False)
                nc.tensor.matmul(sp3[:, 0:n], lhsT=pslice(2 * j),
                                 rhs=v[:, lo:hi, 2 * q - 1:2 * q],
                                 start=False, stop=False)
                nc.tensor.matmul(sp3[:, (lo1 - lo):(lo1 - lo) + n1],
                                 lhsT=pslice(LEVELS + j),
                                 rhs=v[:, lo1 - 1:lo1 - 1 + n1, g4 - 1:g4],
                                 start=False, stop=True)
                bk_add(sp1, v[:, lo1:lo1 + n1, q - 1:q], n1)
                bk_add(sp2, v[:, lo:hi, 2 * q - 1:2 * q], n)
                bk_add(sp3, v[:, lo:hi, 3 * q - 1:3 * q], n)

            def emit_split(blk):
                csl = slice(blk * BLK, (blk + 1) * BLK)
                nc.scalar.copy(SHL[:, csl], wb[:, csl])

            with tc.tile_pool(name="gtmp", bufs=2) as gtpool:
                # one y tile covering all blocks -> a single 128-descriptor
                # (32 KiB each) output DMA at the end
                ysb = ypool.tile([ROWS, SEQ_PER_CORE * COLS], BF16,
                                 tag="ysb", name="ysb")

                def emit_gamma(sq):
                    # seq-outer: 4 consecutive matmuls share the same Gamma
                    # stationary operand
                    for i in range(NBLK):
                        isl = slice(i * BLK, (i + 1) * BLK)
                        ytp = ytpsum.tile([128, BLK], F32, tag="ytp")
                        nc.tensor.matmul(
                            ytp[:],
                            lhsT=gammaT_sb[:, sq * L:(sq + 1) * L],
                            rhs=SHL[:, isl], start=True, stop=True)
                        yband = ysb[:, (i * SEQ_PER_CORE + sq) * BLK:
                                    (i * SEQ_PER_CORE + sq + 1) * BLK]
                        if (i * SEQ_PER_CORE + sq) % 2 == 0:
                            nc.vector.tensor_add(out=yband, in0=ytp[:],
                                                 in1=xsl_yx(sq, i))
                        else:
                            # spread the PSUM drain over Act + Pool
                            gt = gtpool.tile([128, BLK], F32, tag="gt")
                            nc.scalar.copy(gt[:], ytp[:])
                            nc.gpsimd.tensor_add(out=yband, in0=gt[:],
                                                 in1=xsl_yx(sq, i))

                # ---- radix-4 Brent-Kung up-sweep + radix-2 top ----
                # fx jobs drain EARLY (by mid-down-sweep): the next rep's x
                # DMA has a WAR on XT against the last fx read, so draining
                # early lets the following pass's input transfer overlap
                # this pass's scan tail + Gamma phase
                for j in range(RJ):
                    up_level(j)
                    emit_fx(3 if j > 0 else 0)
                vt = stview(NCH)
                spt = spsum.tile([96, BLK], F32, tag="sp")
                nc.tensor.matmul(spt[:, 0:1], lhsT=pslice(10),
                                 rhs=vt[:, 0:1, NCH // 2 - 1:NCH // 2],
                                 start=True, stop=True)
                bk_add(spt, vt[:, 0:1, NCH - 1:NCH], 1)
                emit_fx(2)

                # ---- down-sweep (last level in halves for early Gamma) ----
                for j in range(RJ - 1, 0, -1):
                    down_level(j)
                    emit_fx(5)
                down_level(0, half=0)      # finalizes W[1..1024]
                emit_split(0)
                emit_split(1)
                down_level(0, half=1)
                emit_fx(len(fx_jobs))      # flush any remaining Phi x work
                emit_split(2)
                emit_split(3)
                if ablate == "AS":
                    return
                for sq in range(SEQ_PER_CORE):
                    emit_gamma(sq)
                nc.sync.dma_start(out=y_d[:], in_=ysb[:])


# ----------------------------------------------------------------- entry point
class BassRunner:
    """Builds the sharded jitted executable for a compiled Bass module once;
    subsequent calls only device_put inputs and execute."""

    def __init__(self, nc, n_cores=N_CORES):
        import jax
        from jax.experimental.shard_map import shard_map
        from jax.sharding import Mesh, PartitionSpec
        from concourse.bass2jax import (_bass_exec_p, install_neuronx_cc_hook,
                                        partition_id_tensor)
        install_neuronx_cc_hook()
        self.jax = jax
        partition_name = (nc.partition_id_tensor.name
                          if nc.partition_id_tensor else None)
        in_names, out_names, out_avals, zero_outs = [], [], [], []
        for alloc in nc.m.functions[0].allocations:
            if not isinstance(alloc, mybir.MemoryLocationSet):
                continue
            name = alloc.memorylocations[0].name
            if alloc.kind == "ExternalInput":
                if name != partition_name:
                    in_names.append(name)
            elif alloc.kind == "ExternalOutput":
                out_names.append(name)
                shape = tuple(alloc.tensor_shape)
                dtype = mybir.dt.np(alloc.dtype)
                out_avals.append(jax.core.ShapedArray(shape, dtype))
                zero_outs.append(np.zeros(shape, dtype))
        self.in_names, self.out_names = in_names, out_names
        self.out_avals, self.zero_outs = out_avals, zero_outs
        all_in_names = list(in_names) + list(out_names)
        if partition_name is not None:
            all_in_names.append(partition_name)

        def _body(*args):
            operands = list(args)
            if partition_name is not None:
                operands.append(partition_id_tensor())
            return tuple(_bass_exec_p.bind(
                *operands, out_avals=tuple(out_avals),
                in_names=tuple(all_in_names), out_names=tuple(out_names),
                lowering_input_output_aliases=(),
                sim_require_finite=True, sim_require_nnan=True, nc=nc))

        devices = jax.devices()[:n_cores]
        mesh = Mesh(np.asarray(devices), ("core",))
        nin = len(in_names) + len(out_names)
        self.fn = jax.jit(
            shard_map(_body, mesh=mesh,
                      in_specs=(PartitionSpec("core"),) * nin,
                      out_specs=(PartitionSpec("core"),) * len(out_names),
                      check_rep=False),
            keep_unused=True)
        self.n_cores = n_cores

    def concat_args(self, in_maps):
        args = [np.concatenate([np.asarray(in_maps[c][nm])
                                for c in range(self.n_cores)], axis=0)
                for nm in self.in_names]
        args += [np.zeros((self.n_cores * z.shape[0], *z.shape[1:]), z.dtype)
                 for z in self.zero_outs]
        return args

    def __call__(self, in_maps):
        outs = self.fn(*self.concat_args(in_maps))
        self.jax.block_until_ready(outs)
        return outs


_RUNNER_CACHE = {}


def _get_runner(rep=1):
    if rep not in _RUNNER_CACHE:
        _RUNNER_CACHE[rep] = BassRunner(build_nc(rep=rep))
    return _RUNNER_CACHE[rep]


def _prepare_in_maps(x, p, W, b, sample_rate):
    import ml_dtypes
    bc, ac = _coeffs_from_inputs(p, W, b, sample_rate)
    A, Bv, Cv, D = _state_space(bc, ac)
    h, Gam, M, Pd = _derived(A, Bv, Cv, D)
    toepT, gammaT, mT, scanP = _pack_weights(h, Gam, M, Pd)
    toepT_h = toepT.astype(np.float16)            # (nb, 128, 128)
    gammaT_h = gammaT.astype(np.float16)          # (nb, 96, 128)
    mT_h = mT.astype(np.float16)                  # (nb, 128, 96)
    # chunk-column layout xt[s][m, c] = x[s, c*128 + m], then packed
    # block-major [blk, row, seq, col] per core for contiguous 8 KiB DMAs
    x4 = x.reshape(B * C, NCH, L).astype(np.float32)
    xt = x4.transpose(0, 2, 1)                        # (nb, 128, 2048)
    xs_h = xt.astype(np.float16)
    in_maps = []
    for core in range(N_CORES):
        sl = slice(core * SEQ_PER_CORE, (core + 1) * SEQ_PER_CORE)
        xpk = np.ascontiguousarray(
            xs_h[sl].reshape(SEQ_PER_CORE, ROWS, NBLK, BLK)
            .transpose(1, 2, 0, 3))                   # (row, blk, seq, col)
        # all fp16 weights in one [128, 2816] pack (exact SBUF layout):
        # [0:768) mT, [768:1792) toepT, [1792:2816) gammaT
        # (gammaT rows 96..128 zero)
        wpk = np.zeros((ROWS, 2816), np.float16)
        wpk[:, 0:768] = mT_h[sl].transpose(1, 0, 2).reshape(L, -1)
        wpk[:, 768:1792] = toepT_h[sl].transpose(1, 0, 2).reshape(L, -1)
        wpk[0:96, 1792:2816] = (
            gammaT_h[sl].transpose(1, 0, 2).reshape(96, -1))
        in_maps.append({
            "xh": xpk,
            "wpk": wpk,
            "scanP": np.ascontiguousarray(
                scanP[core].transpose(1, 0, 2).reshape(96, -1)),
        })
    return in_maps


def unpack_y(ypk_all):
    """(n_cores*ROWS, NBLK, SEQ, BLK) packed bf16 -> (B, C, T) fp32."""
    ypk = np.asarray(ypk_all).reshape(N_CORES, ROWS, NBLK, SEQ_PER_CORE, BLK)
    # yq[s][m, blk*512+j] = ypk[m, blk, s, j];  y[s, c*128+m] = yq[s][m, c]
    yq = ypk.transpose(0, 3, 1, 2, 4).reshape(B * C, ROWS, NCH)
    y = np.ascontiguousarray(yq.transpose(0, 2, 1)).astype(np.float32)
    return y.reshape(B, C, T)


def kernel(x, p, W, b, sample_rate):
    runner = _get_runner(rep=1)
    in_maps = _prepare_in_maps(x, p, W, b, sample_rate)
    outs = runner(in_maps)
    return unpack_y(outs[0])


# revision 57
# speedup vs baseline: 1.0233x; 1.0233x over previous
"""Trainium2 Bass kernel for nn_AutodiffChannel: 6-biquad EQ cascade over
(64, 1, 262144) fp32 audio, data-parallel over 8 NeuronCores.

Algorithm (per sequence, LTI block-state decomposition):
  The 6-stage DF2T biquad cascade is a 12-state linear system
  s' = A s + B x, y = C s + D x.  Split T=262144 into 2048 chunks of
  L=128.  Then per chunk c:
      y_c = Phi x_c + Gamma S_c          (Phi  = 128x128 lower-tri Toeplitz
                                          of the impulse response h[0:128],
                                          Gamma[m,:] = C A^m)
      U_c = M x_c                        (M[:,n] = A^(127-n) B)
      S_c = sum_{j<c} (A^128)^(c-1-j) U_j   (exclusive prefix "state scan")
  The prefix is computed with a Kogge-Stone scan (11 levels) using
  precomputed powers P_d = (A^128)^(2^d).  The tiny per-sequence setup
  (h, Gamma, M, P_d) is computed host-side in float64.

Device dataflow per core (8 sequences), lean-precision variant:
  x arrives as bf16 (hi part only; ~2e-3 relative error, inside the 2e-2
  budget).  U = M x uses a 2-term bf16 weight split and accumulates all 8
  seqs into one 96-row PSUM tile per 512-column block.  The prefix scan
  stays fp32 and runs as a radix-4 Brent-Kung (up/down sweeps, ~2N matmul
  columns, 11 serial levels); the scan-independent FIR term Th x is
  spliced into the scan's serial gaps to keep the PE busy and spilled to
  an fp32 SBUF buffer.  S is split into bf16 hi/lo; the correction
  Gamma S uses 3 bf16 terms (Gh Sh, Gl Sh, Gh Sl) fused with the spilled
  FIR part and the bf16 cast in one DVE (or Act+Pool) op.  Total rel err
  ~3.9e-3.  x/y/weights each move in a single DMA with 32 KiB contiguous
  DRAM per partition row (DMA descriptor rate, ~0.6 us/descriptor/queue,
  dominates small transfers).  y returns in chunk-column layout as bf16;
  the host does the final (free) transpose + fp32 cast.
"""
import sys

for _p in ("/opt/trn_rl_repo", "/opt/trn_rl_repo/concourse"):
    if _p not in sys.path:
        sys.path.insert(0, _p)

import numpy as np

import concourse.bacc as bacc
import concourse.mybir as mybir
from concourse.tile import TileContext
from concourse.bass_utils import run_bass_kernel_spmd  # noqa: F401 (env check)

# ---------------------------------------------------------------- problem dims
B, C, T = 64, 1, 262144
N_CORES = 8
SEQ_PER_CORE = B * C // N_CORES  # 8
L = 128                     # chunk length
NCH = T // L                # 2048 chunks per sequence
ROWS = 128                  # partitions: within-chunk sample index
COLS = NCH                  # 2048 chunk columns
LEVELS = 11                 # ceil(log2(NCH))
NSTATE = 12
BLK = 512                   # column blocking (1 PSUM bank of fp32)
NBLK = COLS // BLK
RJ = 5                      # radix-4 scan levels (4^5 = 1024, x2 top)
NSL = LEVELS + RJ           # scanP slots: 11 powers of 2 + 5 "3*4^j" powers
F32 = mybir.dt.float32
BF16 = mybir.dt.bfloat16
F16 = mybir.dt.float16

PARAM_RANGES = np.array([
    [-24.0, 24.0], [20.0, 200.0], [0.1, 10.0],
    [-24.0, 24.0], [200.0, 2000.0], [0.1, 10.0],
    [-24.0, 24.0], [200.0, 2000.0], [0.1, 10.0],
    [-24.0, 24.0], [2000.0, 8000.0], [0.1, 10.0],
    [-24.0, 24.0], [4000.0, 12000.0], [0.1, 10.0],
    [-24.0, 24.0], [4000.0, 12000.0], [0.1, 10.0],
], dtype=np.float32)
FILTER_TYPES = ["low_shelf", "peaking", "peaking", "peaking", "peaking",
                "high_shelf"]


# ------------------------------------------------------------- host-side setup
def _sigmoid_f32(z):
    z = z.astype(np.float32)
    out = np.empty_like(z)
    pos = z >= 0
    out[pos] = (np.float32(1.0) / (np.float32(1.0) + np.exp(-z[pos]))).astype(
        np.float32)
    ez = np.exp(z[~pos]).astype(np.float32)
    out[~pos] = (ez / (np.float32(1.0) + ez)).astype(np.float32)
    return out


def _biquad_coeffs_f32(g, f, q, sr, ftype):
    """fp32-faithful audio-EQ-cookbook coefficients (matches reference)."""
    f32 = np.float32
    A = np.power(f32(10.0), (g / f32(40.0)).astype(f32)).astype(f32)
    w0 = (f32(2.0) * f32(np.pi) * (f / f32(sr))).astype(f32)
    alpha = (np.sin(w0, dtype=f32) / (f32(2.0) * q)).astype(f32)
    c = np.cos(w0, dtype=f32)
    sA = np.sqrt(A).astype(f32)
    one, two = f32(1.0), f32(2.0)
    if ftype == "low_shelf":
        b0 = A * ((A + one) - (A - one) * c + two * sA * alpha)
        b1 = two * A * ((A - one) - (A + one) * c)
        b2 = A * ((A + one) - (A - one) * c - two * sA * alpha)
        a0 = (A + one) + (A - one) * c + two * sA * alpha
        a1 = -two * ((A - one) + (A + one) * c)
        a2 = (A + one) + (A - one) * c - two * sA * alpha
    elif ftype == "high_shelf":
        b0 = A * ((A + one) + (A - one) * c + two * sA * alpha)
        b1 = -two * A * ((A - one) + (A + one) * c)
        b2 = A * ((A + one) + (A - one) * c - two * sA * alpha)
        a0 = (A + one) - (A - one) * c + two * sA * alpha
        a1 = two * ((A - one) - (A + one) * c)
        a2 = (A + one) - (A - one) * c - two * sA * alpha
    else:
        b0 = one + alpha * A
        b1 = -two * c
        b2 = one - alpha * A
        a0 = one + alpha / A
        a1 = -two * c
        a2 = one - alpha / A
    bc = (np.stack([b0, b1, b2], -1).astype(f32) / a0[..., None]).astype(f32)
    ac = (np.stack([a0, a1, a2], -1).astype(f32) / a0[..., None]).astype(f32)
    return bc, ac


def _coeffs_from_inputs(p, W, b, sample_rate):
    z = (p.astype(np.float32) @ W.astype(np.float32).T
         + b.astype(np.float32)).astype(np.float32)
    pn = _sigmoid_f32(z)
    lo, hi = PARAM_RANGES[:, 0], PARAM_RANGES[:, 1]
    params = (pn * (hi - lo) + lo).astype(np.float32)
    bcs, acs = [], []
    for k, ftype in enumerate(FILTER_TYPES):
        bc, ac = _biquad_coeffs_f32(
            params[:, 3 * k], params[:, 3 * k + 1], params[:, 3 * k + 2],
            float(sample_rate), ftype)
        bcs.append(bc)
        acs.append(ac)
    return np.stack(bcs), np.stack(acs)  # (6, B, 3) fp32


def _state_space(bc, ac):
    """Vectorized float64 (A, B, C, D) per sequence from fp32 DF2T coeffs."""
    nb = bc.shape[1]
    bc64 = bc.astype(np.float64)
    ac64 = ac.astype(np.float64)

    def step(s, x):
        s = s.copy()
        v = x
        for k in range(6):
            b0, b1, b2 = bc64[k, :, 0], bc64[k, :, 1], bc64[k, :, 2]
            a1, a2 = ac64[k, :, 1], ac64[k, :, 2]
            s1, s2 = s[:, 2 * k], s[:, 2 * k + 1]
            y = b0 * v + s1
            s[:, 2 * k] = b1 * v - a1 * y + s2
            s[:, 2 * k + 1] = b2 * v - a2 * y
            v = y
        return s, v

    A = np.zeros((nb, NSTATE, NSTATE))
    Cv = np.zeros((nb, NSTATE))
    for i in range(NSTATE):
        e = np.zeros((nb, NSTATE))
        e[:, i] = 1.0
        sp, y = step(e, np.zeros(nb))
        A[:, :, i] = sp
        Cv[:, i] = y
    Bv, D = step(np.zeros((nb, NSTATE)), np.ones(nb))
    return A, Bv, Cv, D


def _derived(A, Bv, Cv, D):
    """h (nb,L), Gamma (nb,L,12), M (nb,12,L), Pd (nb,LEVELS,12,12) in f64."""
    nb = A.shape[0]
    h = np.zeros((nb, L))
    Gam = np.zeros((nb, L, NSTATE))
    M = np.zeros((nb, NSTATE, L))
    h[:, 0] = D
    cam = Cv.copy()          # C A^m
    amb = Bv.copy()          # A^m B
    for m in range(L):
        Gam[:, m, :] = cam
        M[:, :, L - 1 - m] = amb
        if m + 1 < L:
            h[:, m + 1] = np.einsum("bi,bi->b", cam, Bv)
        cam = np.einsum("bi,bij->bj", cam, A)
        amb = np.einsum("bij,bj->bi", A, amb)
    sq = A.copy()
    for _ in range(7):       # A^(2^7) = A^128
        sq = sq @ sq
    Pd = np.zeros((nb, NSL, NSTATE, NSTATE))
    for d in range(LEVELS):
        Pd[:, d] = sq
        sq = sq @ sq
    for j in range(RJ):      # (A^128)^(3*4^j) for the radix-4 scan
        Pd[:, LEVELS + j] = Pd[:, 2 * j] @ Pd[:, 2 * j + 1]
    return h, Gam, M, Pd


def _split_hi_lo(a):
    """Split fp32 into bf16 hi + bf16 lo (a ~= hi + lo, ~17-bit mantissa)."""
    import ml_dtypes
    a = a.astype(np.float32)
    hi = a.astype(ml_dtypes.bfloat16)
    lo = (a - hi.astype(np.float32)).astype(ml_dtypes.bfloat16)
    return hi, lo


def _pack_weights(h, Gam, M, Pd):
    """fp32 device weight tensors, per core."""
    nb = h.shape[0]
    m_idx = np.arange(L)
    diff = m_idx[None, :] - m_idx[:, None]          # [n, m] = m - n
    toepT = np.where(diff >= 0, h[:, np.clip(diff, 0, L - 1)],
                     0.0).astype(np.float32)        # (nb, n=128, m=128)
    # mT/gammaT embedded at per-seq 12-row offsets inside a 96-row frame so
    # every device access stays at base partition 0
    gammaT = np.zeros((nb, 96, L), np.float32)         # (nb, k-embed, m)
    mT = np.zeros((nb, L, 96), np.float32)             # (nb, n, k-embed)
    for g in range(nb):
        s8 = g % SEQ_PER_CORE
        gammaT[g, 12 * s8:12 * s8 + 12, :] = Gam[g].T.astype(np.float32)
        mT[g, :, 12 * s8:12 * s8 + 12] = M[g].T.astype(np.float32)
    scanP = np.zeros((N_CORES, NSL, 96, 96), np.float32)
    for core in range(N_CORES):
        for s in range(SEQ_PER_CORE):
            g = core * SEQ_PER_CORE + s
            for d in range(NSL):
                scanP[core, d, 12 * s:12 * s + 12, 12 * s:12 * s + 12] = \
                    Pd[g, d].T.astype(np.float32)
    return toepT, gammaT, mT, scanP


# ------------------------------------------------------------ device kernel IR
_NC_CACHE = {}


def build_nc(rep=1, ablate=""):
    key = (rep, ablate)
    if key in _NC_CACHE:
        return _NC_CACHE[key]
    nc = bacc.Bacc("TRN2")
    # DMA descriptor rate (~0.6us per descriptor per queue) dominates small
    # transfers, so x/y/bf16-weights each use ONE dma with 32 KiB contiguous
    # DRAM per partition row: layout [row, blk, seq, col]
    xh_d = nc.dram_tensor("xh", [ROWS, NBLK, SEQ_PER_CORE, BLK], F16,
                          kind="ExternalInput")
    # packed fp16 weights (single terms): cols [0:768) mT, [768:1792)
    # toepT, [1792:2816) gammaT (rows 96..128 zero-padded)
    wpk_d = nc.dram_tensor("wpk", [ROWS, 2816], F16, kind="ExternalInput")
    scanP_d = nc.dram_tensor("scanP", [96, NSL * 96], F32,
                             kind="ExternalInput")
    y_d = nc.dram_tensor("y", [ROWS, NBLK, SEQ_PER_CORE, BLK], BF16,
                         kind="ExternalOutput")

    with TileContext(nc) as tc:
        with tc.tile_pool(name="weights", bufs=1) as wpool:
            wpk_sb = wpool.tile([ROWS, 2816], F16)
            scanP_sb = wpool.tile([96, NSL * 96], F32)
            mT_sb = wpk_sb[:, 0:768]
            toepT_sb = wpk_sb[:, 768:1792]
            gammaT_sb = wpk_sb[0:96, 1792:2816]

            def dma_weights():
                nc.sync.dma_start(out=wpk_sb, in_=wpk_d[:])
                nc.sync.dma_start(out=scanP_sb, in_=scanP_d[:])

            with tc.tile_pool(name="xt", bufs=1) as xtpool, \
                 tc.tile_pool(name="ysb", bufs=1) as ypool:
                for r in range(rep):
                    _one_pass(nc, tc, xh_d, y_d, toepT_sb, gammaT_sb,
                              mT_sb, scanP_sb, xtpool, ypool, ablate,
                              dma_weights if r == 0 else None)
    nc.compile()
    _NC_CACHE[key] = nc
    return nc


def _one_pass(nc, tc, xh_d, y_d, toepT_sb, gammaT_sb, mT_sb, scanP_sb,
              xtpool, ypool, ablate="", dma_weights=None):
    # single x tile, column index = blk*4096 + sq*512 + j; two half DMAs
    # (16 KiB per partition row) so phase A starts after the first half
    XT = xtpool.tile([ROWS, SEQ_PER_CORE * COLS], F16, tag="xt", name="xt")
    nc.sync.dma_start(out=XT[:, 0:2 * 8 * BLK], in_=xh_d[:, 0:2])
    nc.sync.dma_start(out=XT[:, 2 * 8 * BLK:], in_=xh_d[:, 2:4])
    if dma_weights is not None:
        dma_weights()

    def xsl(sq, i):
        off = (i * SEQ_PER_CORE + sq) * BLK
        return XT[:, off:off + BLK]

    def wsl(base, h, sq, width):     # hi/lo weight slice helper
        off = (h * SEQ_PER_CORE + sq) * width
        return base[:, off:off + width]

    with tc.tile_pool(name="wbuf", bufs=1) as wbpool:
        # scan state (in place): col 0 = zeros, col 1+c = U_c then the
        # inclusive prefix W_c; rows 12s..12s+12 = seq s
        wb = wbpool.tile([96, NCH + 1], F32, tag="wb")
        # fp32 spill of the scan-independent FIR part Phi x (per seq)
        yx = wbpool.tile([ROWS, SEQ_PER_CORE * COLS], F32, tag="yx")
        nc.gpsimd.memset(wb[:, 0:1], 0.0)

        # ---- phase A: U_c = M x_c, all 8 seqs into one 96-row tile ----
        with tc.tile_pool(name="up", bufs=2, space="PSUM") as upsum:
            for i in range(NBLK):
                up = upsum.tile([96, BLK], F32, tag="up")
                for sq in range(SEQ_PER_CORE):
                        nc.tensor.matmul(
                            up[:], lhsT=mT_sb[:, sq * 96:(sq + 1) * 96],
                            rhs=xsl(sq, i),
                            start=(sq == 0), stop=(sq == SEQ_PER_CORE - 1))
                # Pool/GpSimd cannot read PSUM: copies must use DVE or Act
                if i % 2 == 0:
                    nc.vector.tensor_copy(
                        out=wb[:, 1 + i * BLK:1 + (i + 1) * BLK], in_=up[:])
                else:
                    nc.scalar.copy(wb[:, 1 + i * BLK:1 + (i + 1) * BLK],
                                   up[:])

        if ablate == "A":
            return

        with tc.tile_pool(name="shl", bufs=1) as shlpool, \
             tc.tile_pool(name="sp", bufs=4, space="PSUM") as spsum, \
             tc.tile_pool(name="ytp", bufs=4, space="PSUM") as ytpsum:
            # fp16 S (single rounding; fp16 range covers |S| ~ 3e3)
            SHL = shlpool.tile([96, NCH], F16, tag="shl")
            # wb cols 1..2048 as a scan array W[1..N]
            wbv = wb[:, 1:NCH + 1]

            fx_jobs = [(sq, i) for i in range(NBLK)
                       for sq in range(SEQ_PER_CORE)]
            fx_done = [0]

            def xsl_yx(sq, i):
                off = (i * SEQ_PER_CORE + sq) * BLK
                return yx[:, off:off + BLK]

            def emit_fx(n):
                # Phi x term (Th only): independent of the scan; spliced
                # into the Brent-Kung serial gaps to keep the PE busy
                for _ in range(n):
                    if fx_done[0] >= len(fx_jobs):
                        return
                    sq, i = fx_jobs[fx_done[0]]
                    fx_done[0] += 1
                    ytp = ytpsum.tile([128, BLK], F32, tag="ytp")
                    nc.tensor.matmul(
                        ytp[:], lhsT=toepT_sb[:, sq * L:(sq + 1) * L],
                        rhs=xsl(sq, i), start=True, stop=True)
                    if (sq + i) % 2 == 0:
                        nc.vector.tensor_copy(out=xsl_yx(sq, i), in_=ytp[:])
                    else:
                        nc.scalar.copy(xsl_yx(sq, i), ytp[:])

            def pslice(d):
                return scanP_sb[:, d * 96:(d + 1) * 96]

            def stview(stride):
                return wbv.rearrange("p (k s) -> p k s", s=stride)

            def bk_add(sp, tgt, nk):
                nc.vector.tensor_add(
                    out=tgt, in0=tgt,
                    in1=sp[:, 0:nk].rearrange("p (n o) -> p n o", o=1))

            def up_level(j):
                # radix-4: W[g4Q+4Q] += P^3Q W[g4Q+Q] + P^2Q W[g4Q+2Q]
                #                       + P^Q W[g4Q+3Q]
                q = 4 ** j
                g4 = 4 * q
                ng = NCH // g4
                v = stview(g4)
                sp = spsum.tile([96, BLK], F32, tag="sp")
                srcs = ((LEVELS + j, q - 1), (2 * j + 1, 2 * q - 1),
                        (2 * j, 3 * q - 1))
                for k, (d, idx) in enumerate(srcs):
                    nc.tensor.matmul(sp[:, 0:ng], lhsT=pslice(d),
                                     rhs=v[:, 0:ng, idx:idx + 1],
                                     start=(k == 0), stop=(k == 2))
                bk_add(sp, v[:, 0:ng, g4 - 1:g4], ng)

            def down_level(j, half=None):
                # finalize W[g4Q+kQ] (k=1,2,3) from the complete prefix
                # W[g4Q] (g>=1) + the old Q-block partials
                q = 4 ** j
                g4 = 4 * q
                ng = NCH // g4
                v = stview(g4)
                lo, hi = 0, ng
                if half == 0:
                    hi = ng // 2
                elif half == 1:
                    lo = ng // 2
                n = hi - lo
                # ALL matmuls first (they must see pre-level "old" Q-block
                # partials), THEN the adds: emitting an add earlier would
                # make the later matmuls read the already-updated columns
                n1, lo1 = (n - 1, lo + 1) if lo == 0 else (n, lo)
                # k=1: src complete W[g4Q] only (g >= 1)
                sp1 = spsum.tile([96, BLK], F32, tag="sp")
                nc.tensor.matmul(sp1[:, 0:n1], lhsT=pslice(2 * j),
                                 rhs=v[:, lo1 - 1:lo1 - 1 + n1,
                                       g4 - 1:g4],
                                 start=True, stop=True)
                # k=2: P^2Q W[g4Q] (g>=1) + P^Q W_old[g4Q+Q]
                sp2 = spsum.tile([96, BLK], F32, tag="sp")
                nc.tensor.matmul(sp2[:, 0:n], lhsT=pslice(2 * j),
                                 rhs=v[:, lo:hi, q - 1:q],
                                 start=True, stop=False)
                nc.tensor.matmul(sp2[:, (lo1 - lo):(lo1 - lo) + n1],
                                 lhsT=pslice(2 * j + 1),
                                 rhs=v[:, lo1 - 1:lo1 - 1 + n1, g4 - 1:g4],
                                 start=False, stop=True)
                # k=3: P^3Q W[g4Q] (g>=1) + P^2Q W_old[+Q] + P^Q W_old[+2Q]
                sp3 = spsum.tile([96, BLK], F32, tag="sp")
                nc.tensor.matmul(sp3[:, 0:n], lhsT=pslice(2 * j + 1),
                                 rhs=v[:, lo:hi, q - 1:q],
                                 start=True, stop=False)
                nc.tensor.matmul(sp3[:, 0:n], lhsT=pslice(2 * j),
                                 rhs=v[:, lo:hi, 2 * q - 1:2 * q],
                                 start=False, stop=False)
                nc.tensor.matmul(sp3[:, (lo1 - lo):(lo1 - lo) + n1],
                                 lhsT=pslice(LEVELS + j),
                                 rhs=v[:, lo1 - 1:lo1 - 1 + n1, g4 - 1:g4],
                                 start=False, stop=True)
                bk_add(sp1, v[:, lo1:lo1 + n1, q - 1:q], n1)
                bk_add(sp2, v[:, lo:hi, 2 * q - 1:2 * q], n)
                bk_add(sp3, v[:, lo:hi, 3 * q - 1:3 * q], n)

            def emit_split(blk):
                csl = slice(blk * BLK, (blk + 1) * BLK)
                nc.scalar.copy(SHL[:, csl], wb[:, csl])

            with tc.tile_pool(name="gtmp", bufs=3) as gtpool:
                # one y tile covering all blocks -> a single 128-descriptor
                # (32 KiB each) output DMA at the end
                ysb = ypool.tile([ROWS, SEQ_PER_CORE * COLS], BF16,
                                 tag="ysb", name="ysb")

                def emit_gamma(i):
                    isl = slice(i * BLK, (i + 1) * BLK)
                    for sq in range(SEQ_PER_CORE):
                        ytp = ytpsum.tile([128, BLK], F32, tag="ytp")
                        nc.tensor.matmul(
                            ytp[:],
                            lhsT=gammaT_sb[:, sq * L:(sq + 1) * L],
                            rhs=SHL[:, isl], start=True, stop=True)
                        yband = ysb[:, (i * SEQ_PER_CORE + sq) * BLK:
                                    (i * SEQ_PER_CORE + sq + 1) * BLK]
                        if (i * SEQ_PER_CORE + sq) % 2 == 0:
                            nc.vector.tensor_add(out=yband, in0=ytp[:],
                                                 in1=xsl_yx(sq, i))
                        else:
                            # spread the PSUM drain over Act + Pool
                            gt = gtpool.tile([128, BLK], F32, tag="gt")
                            nc.scalar.copy(gt[:], ytp[:])
                            nc.gpsimd.tensor_add(out=yband, in0=gt[:],
                                                 in1=xsl_yx(sq, i))

                # ---- radix-4 Brent-Kung up-sweep + radix-2 top ----
                # fx jobs drain EARLY (by mid-down-sweep): the next rep's x
                # DMA has a WAR on XT against the last fx read, so draining
                # early lets the following pass's input transfer overlap
                # this pass's scan tail + Gamma phase
                for j in range(RJ):
                    up_level(j)
                    emit_fx(3 if j > 0 else 0)
                vt = stview(NCH)
                spt = spsum.tile([96, BLK], F32, tag="sp")
                nc.tensor.matmul(spt[:, 0:1], lhsT=pslice(10),
                                 rhs=vt[:, 0:1, NCH // 2 - 1:NCH // 2],
                                 start=True, stop=True)
                bk_add(spt, vt[:, 0:1, NCH - 1:NCH], 1)
                emit_fx(2)

                # ---- down-sweep (last level in halves for early Gamma) ----
                for j in range(RJ - 1, 0, -1):
                    down_level(j)
                    emit_fx(5)
                down_level(0, half=0)      # finalizes W[1..1024]
                emit_split(0)
                emit_split(1)
                down_level(0, half=1)
                emit_fx(len(fx_jobs))      # flush any remaining Phi x work
                emit_split(2)
                emit_split(3)
                if ablate == "AS":
                    return
                emit_gamma(0)
                emit_gamma(1)
                emit_gamma(2)
                emit_gamma(3)
                nc.sync.dma_start(out=y_d[:], in_=ysb[:])


# ----------------------------------------------------------------- entry point
class BassRunner:
    """Builds the sharded jitted executable for a compiled Bass module once;
    subsequent calls only device_put inputs and execute."""

    def __init__(self, nc, n_cores=N_CORES):
        import jax
        from jax.experimental.shard_map import shard_map
        from jax.sharding import Mesh, PartitionSpec
        from concourse.bass2jax import (_bass_exec_p, install_neuronx_cc_hook,
                                        partition_id_tensor)
        install_neuronx_cc_hook()
        self.jax = jax
        partition_name = (nc.partition_id_tensor.name
                          if nc.partition_id_tensor else None)
        in_names, out_names, out_avals, zero_outs = [], [], [], []
        for alloc in nc.m.functions[0].allocations:
            if not isinstance(alloc, mybir.MemoryLocationSet):
                continue
            name = alloc.memorylocations[0].name
            if alloc.kind == "ExternalInput":
                if name != partition_name:
                    in_names.append(name)
            elif alloc.kind == "ExternalOutput":
                out_names.append(name)
                shape = tuple(alloc.tensor_shape)
                dtype = mybir.dt.np(alloc.dtype)
                out_avals.append(jax.core.ShapedArray(shape, dtype))
                zero_outs.append(np.zeros(shape, dtype))
        self.in_names, self.out_names = in_names, out_names
        self.out_avals, self.zero_outs = out_avals, zero_outs
        all_in_names = list(in_names) + list(out_names)
        if partition_name is not None:
            all_in_names.append(partition_name)

        def _body(*args):
            operands = list(args)
            if partition_name is not None:
                operands.append(partition_id_tensor())
            return tuple(_bass_exec_p.bind(
                *operands, out_avals=tuple(out_avals),
                in_names=tuple(all_in_names), out_names=tuple(out_names),
                lowering_input_output_aliases=(),
                sim_require_finite=True, sim_require_nnan=True, nc=nc))

        devices = jax.devices()[:n_cores]
        mesh = Mesh(np.asarray(devices), ("core",))
        nin = len(in_names) + len(out_names)
        self.fn = jax.jit(
            shard_map(_body, mesh=mesh,
                      in_specs=(PartitionSpec("core"),) * nin,
                      out_specs=(PartitionSpec("core"),) * len(out_names),
                      check_rep=False),
            keep_unused=True)
        self.n_cores = n_cores

    def concat_args(self, in_maps):
        args = [np.concatenate([np.asarray(in_maps[c][nm])
                                for c in range(self.n_cores)], axis=0)
                for nm in self.in_names]
        args += [np.zeros((self.n_cores * z.shape[0], *z.shape[1:]), z.dtype)
                 for z in self.zero_outs]
        return args

    def __call__(self, in_maps):
        outs = self.fn(*self.concat_args(in_maps))
        self.jax.block_until_ready(outs)
        return outs


_RUNNER_CACHE = {}


def _get_runner(rep=1):
    if rep not in _RUNNER_CACHE:
        _RUNNER_CACHE[rep] = BassRunner(build_nc(rep=rep))
    return _RUNNER_CACHE[rep]


def _prepare_in_maps(x, p, W, b, sample_rate):
    import ml_dtypes
    bc, ac = _coeffs_from_inputs(p, W, b, sample_rate)
    A, Bv, Cv, D = _state_space(bc, ac)
    h, Gam, M, Pd = _derived(A, Bv, Cv, D)
    toepT, gammaT, mT, scanP = _pack_weights(h, Gam, M, Pd)
    toepT_h = toepT.astype(np.float16)            # (nb, 128, 128)
    gammaT_h = gammaT.astype(np.float16)          # (nb, 96, 128)
    mT_h = mT.astype(np.float16)                  # (nb, 128, 96)
    # chunk-column layout xt[s][m, c] = x[s, c*128 + m], then packed
    # block-major [blk, row, seq, col] per core for contiguous 8 KiB DMAs
    x4 = x.reshape(B * C, NCH, L).astype(np.float32)
    xt = x4.transpose(0, 2, 1)                        # (nb, 128, 2048)
    xs_h = xt.astype(np.float16)
    in_maps = []
    for core in range(N_CORES):
        sl = slice(core * SEQ_PER_CORE, (core + 1) * SEQ_PER_CORE)
        xpk = np.ascontiguousarray(
            xs_h[sl].reshape(SEQ_PER_CORE, ROWS, NBLK, BLK)
            .transpose(1, 2, 0, 3))                   # (row, blk, seq, col)
        # all fp16 weights in one [128, 2816] pack (exact SBUF layout):
        # [0:768) mT, [768:1792) toepT, [1792:2816) gammaT
        # (gammaT rows 96..128 zero)
        wpk = np.zeros((ROWS, 2816), np.float16)
        wpk[:, 0:768] = mT_h[sl].transpose(1, 0, 2).reshape(L, -1)
        wpk[:, 768:1792] = toepT_h[sl].transpose(1, 0, 2).reshape(L, -1)
        wpk[0:96, 1792:2816] = (
            gammaT_h[sl].transpose(1, 0, 2).reshape(96, -1))
        in_maps.append({
            "xh": xpk,
            "wpk": wpk,
            "scanP": np.ascontiguousarray(
                scanP[core].transpose(1, 0, 2).reshape(96, -1)),
        })
    return in_maps


def unpack_y(ypk_all):
    """(n_cores*ROWS, NBLK, SEQ, BLK) packed bf16 -> (B, C, T) fp32."""
    ypk = np.asarray(ypk_all).reshape(N_CORES, ROWS, NBLK, SEQ_PER_CORE, BLK)
    # yq[s][m, blk*512+j] = ypk[m, blk, s, j];  y[s, c*128+m] = yq[s][m, c]
    yq = ypk.transpose(0, 3, 1, 2, 4).reshape(B * C, ROWS, NCH)
    y = np.ascontiguousarray(yq.transpose(0, 2, 1)).astype(np.float32)
    return y.reshape(B, C, T)


def kernel(x, p, W, b, sample_rate):
    runner = _get_runner(rep=1)
    in_maps = _prepare_in_maps(x, p, W, b, sample_rate)
    outs = runner(in_maps)
    return unpack_y(outs[0])
